# revision 41
# baseline (speedup 1.0000x reference)
"""Grouped-Query Attention (B=2, S=2048, E=2048, 32 q heads, 8 kv heads, d=64)
on 8 Trainium2 NeuronCores.

Sharding: 8 cores = 2 batches x 4 kv-head-groups. Each core handles one batch
and 2 kv heads (= 8 q heads), computing its slice of attention plus the
row-parallel partial out-projection. The host sums the 4 partial outputs per
batch (no on-device collectives needed) and adds the output bias.

On-device pipeline per core (matmuls bf16, fp32 accumulation):
  warmup: k(0) + v(tt 0..3) + q(n=0, qi=0) et-interleaved against et-granular
  DMA (xT chunk0 on sync, fused weights on scalar) so PE is fed from ~2us ->
  scoresT = k @ qT (t on partitions) -> exp on ScalarE (scale 1/8 folded in,
  no max-subtraction: scores are O(5) for unit-variance inputs) ->
  AV^T: pav[q, d|denom] += exps_tile.T @ [v | ones]  (the ones column gives
  the softmax denominator directly, per (q, head), no cross-partition
  reduction needed) -> reciprocal + normalize (DVE) -> PE transpose back to
  [d, q] -> out-projection, software-pipelined one block behind attention.
  Last iteration: pav banks pre-zeroed by PE zeros-matmuls so the 8 AV
  chains run start-free split into an ungated (tt<12) and a gated (tt>=12)
  part around the out_proj(14) filler; per-head normalize/transpose/out-proj
  pipeline ordered to avoid tile-granular WAR serialization; final output
  pieces drain on DVE+ACT copies and sync+scalar DMA queues.
"""

import sys

sys.path.insert(0, "/opt/trn_rl_repo")

import numpy as np
import ml_dtypes

BF16 = ml_dtypes.bfloat16

P = 128
B, S, E = 2, 2048, 2048
NUM_HEADS, NUM_KV_HEADS, HEAD_DIM = 32, 8, 64
GROUP = NUM_HEADS // NUM_KV_HEADS  # 4
NE = E // P  # 16 e-tiles (contraction tiles for projections)
NT = S // P  # 16 t-tiles (key/value positions)
NJ = GROUP  # 4 q-heads per kv head
SB = 128  # query-block size
NSB = S // SB  # 16 query blocks
NCH = 4  # xT S-chunks
SCALE = 1.0 / np.sqrt(HEAD_DIM)

_compiled = None  # cached program
_RUN_KWARGS = {}  # test harness may set e.g. {"trace": True}
_last_run = None  # BassKernelResults of the most recent kernel() call
PHASE = [""]  # build-time phase label, for the analysis tooling


def build_gqa_program():
    from concourse import bacc, mybir, tile, masks

    f32 = mybir.dt.float32
    bf16 = mybir.dt.bfloat16
    Exp = mybir.ActivationFunctionType.Exp
    Copy = mybir.ActivationFunctionType.Copy

    nc = bacc.Bacc(None, target_bir_lowering=False, debug=False)
    with tile.TileContext(nc) as tc:
        with tc.tile_pool(name="dram", bufs=1, space="DRAM") as dram:
            xT = dram.tile([P, NCH, NE, 512], bf16, kind="ExternalInput", name="xT", uniquify=False)
            # wmix[:, et] = [wk 128 | wv 128 | wq j0..j3 512] for that e-tile
            wmix = dram.tile([P, NE, 768], bf16, kind="ExternalInput", name="wmix", uniquify=False)
            wo = dram.tile([P, NJ, E], bf16, kind="ExternalInput", name="wo", uniquify=False)
            # ball = [bq j0..j3 (per-partition) | bk (per-partition) | bv 128 (free-dim)]
            ball = dram.tile([P, 133], f32, kind="ExternalInput", name="ball", uniquify=False)
            y = dram.tile([P, NT, E], bf16, kind="ExternalOutput", name="y", uniquify=False)

            with (
                tc.tile_pool(name="win", bufs=1) as win,
                tc.tile_pool(name="proj", bufs=1) as proj,
                tc.tile_pool(name="attn", bufs=2) as attn,
                tc.tile_pool(name="misc", bufs=2) as misc,
                tc.tile_pool(name="ps", bufs=2, space="PSUM") as ps,
            ):
                # ---- input DMAs, 3 queues, consumption-ordered. Warmup data
                # (chunk0 + wmix) is split across all three queues; the exp
                # table load pins the scalar queue's head, so the first
                # critical pieces ride sync/gpsimd.
                xT_sb = win.tile([P, NCH, NE, 512], bf16)
                wmix_sb = win.tile([P, NE, 768], bf16)
                ball_sb = win.tile([P, 133], f32)
                wo_sb = win.tile([P, NJ, E], bf16)

                nc.sync.dma_start(out=wmix_sb[:, 0, 0:256], in_=wmix[:, 0, 0:256])
                nc.sync.dma_start(out=xT_sb[:, 0, 1:2], in_=xT[:, 0, 1:2])
                nc.sync.dma_start(out=xT_sb[:, 0, 2:4], in_=xT[:, 0, 2:4])
                nc.sync.dma_start(out=xT_sb[:, 0, 4:6], in_=xT[:, 0, 4:6])
                nc.sync.dma_start(out=xT_sb[:, 0, 6:8], in_=xT[:, 0, 6:8])
                nc.sync.dma_start(out=wmix_sb[:, 11:16], in_=wmix[:, 11:16])
                nc.sync.dma_start(out=ball_sb[:], in_=ball[:])
                nc.sync.dma_start(out=xT_sb[:, 2, 0:8], in_=xT[:, 2, 0:8])
                nc.sync.dma_start(out=xT_sb[:, 2, 8:16], in_=xT[:, 2, 8:16])

                nc.scalar.dma_start(out=wmix_sb[:, 0, 256:768], in_=wmix[:, 0, 256:768])
                nc.scalar.dma_start(out=wmix_sb[:, 1:2], in_=wmix[:, 1:2])
                nc.scalar.dma_start(out=wmix_sb[:, 2:4], in_=wmix[:, 2:4])
                nc.scalar.dma_start(out=wmix_sb[:, 7:11], in_=wmix[:, 7:11])
                nc.scalar.dma_start(out=xT_sb[:, 3, 0:8], in_=xT[:, 3, 0:8])
                nc.scalar.dma_start(out=xT_sb[:, 3, 8:16], in_=xT[:, 3, 8:16])

                nc.gpsimd.dma_start(out=xT_sb[:, 0, 0:1], in_=xT[:, 0, 0:1])
                nc.gpsimd.dma_start(out=xT_sb[:, 0, 8:12], in_=xT[:, 0, 8:12])
                nc.gpsimd.dma_start(out=wmix_sb[:, 4:7], in_=wmix[:, 4:7])
                nc.gpsimd.dma_start(out=xT_sb[:, 0, 12:16], in_=xT[:, 0, 12:16])
                # v with a ones column per kv head: [v_g | 1] -> denominator
                # comes out of the AV matmul for free.
                zrow = win.tile([1, 512], bf16)
                nc.gpsimd.memset(zrow[:], 0.0)
                v_aug = [win.tile([P, NT, 65], bf16, name=f"vaug{g}") for g in range(2)]
                for g in range(2):
                    nc.gpsimd.memset(v_aug[g][:, :, 64:65], 1.0)
                ident = win.tile([P, P], bf16)
                masks.make_identity(nc, ident[:])
                nc.gpsimd.dma_start(out=xT_sb[:, 1, 0:8], in_=xT[:, 1, 0:8])
                nc.gpsimd.dma_start(out=xT_sb[:, 1, 8:16], in_=xT[:, 1, 8:16])
                nc.gpsimd.dma_start(out=wo_sb[:], in_=wo[:])

                def wk_at(et):
                    return wmix_sb[:, et, 0:128]

                def wv_at(et):
                    return wmix_sb[:, et, 128:256]

                def wq_at(et, j):
                    return wmix_sb[:, et, 256 + j * 128:256 + (j + 1) * 128]

                bq_sb = ball_sb[:, 0:NJ]
                bk_sb = ball_sb[:, NJ:NJ + 1]
                bv_sb = ball_sb[:, NJ + 1:NJ + 1 + 128]

                # exp table warm-up; reads wmix so the ACT table load schedules
                # after the warmup-critical DMAs on the scalar queue.
                warm = misc.tile([1, 1], f32, tag="warm")
                nc.scalar.activation(out=warm[:], in_=wmix_sb[0:1, 15, 0:1], func=Exp)

                kT = proj.tile([P, S], bf16)
                qT = proj.tile([P, NJ, S], bf16)

                # ---- warmup: k(0) + v(tt 0..3) + q(n=0, qi=0) et-interleaved.
                # pvpack/pqpack hold 4 accumulation quadrants per bank; the
                # very first matmul's start=True lazily zeroes the whole bank,
                # everything else accumulates start-free.
                PHASE[0] = "warm"
                pk = ps.tile([P, 512], f32, tag="sc", name="pk")
                pvpack = ps.tile([P, 4, 128], f32, tag="sc", name="pvpack")
                pqpack = ps.tile([P, 4, 128], f32, tag="pav0", bufs=1, name="pqpack")

                def warm_q(qet):
                    for j in range(NJ):
                        nc.tensor.matmul(
                            pqpack[:, j], wq_at(qet, j), xT_sb[:, 0, qet, 0:128],
                            start=(qet == 0 and j == 0), stop=(qet == NE - 1),
                            skip_group_check=True,
                        )

                # q lags k/v by two e-tiles: its weights ride the scalar queue
                # behind the exp-table load, and the lag keeps the in-order PE
                # stream from blocking on them.
                for et in range(NE):
                    if et >= 3:
                        warm_q(et - 3)
                    nc.tensor.matmul(
                        pk[:], wk_at(et), xT_sb[:, 0, et],
                        start=(et == 0), stop=(et == NE - 1),
                    )
                    for tt in range(4):
                        nc.tensor.matmul(
                            pvpack[:, tt],
                            xT_sb[:, 0, et, tt * 128:(tt + 1) * 128],
                            wv_at(et),
                            start=(et == 0 and tt == 0), stop=(et == NE - 1),
                            skip_group_check=True,
                        )
                for qet in range(NE - 3, NE):
                    warm_q(qet)
                # pops (DVE): k first (scores h0 needs it), then q, then v.
                nc.vector.tensor_scalar_add(
                    out=kT[:, 0:512], in0=pk[:], scalar1=bk_sb
                )
                for j in range(NJ):
                    nc.vector.tensor_scalar_add(
                        out=qT[:, j, 0:128], in0=pqpack[:, j],
                        scalar1=bq_sb[:, j:j + 1],
                    )
                for tt in range(4):
                    for g in range(2):
                        nc.vector.tensor_add(
                            out=v_aug[g][:, tt, 0:64],
                            in0=pvpack[:, tt, g * 64:(g + 1) * 64],
                            in1=bv_sb[:, g * 64:(g + 1) * 64],
                        )

                # ---- work units ----
                def k_proj(n):
                    pkl = ps.tile([P, 512], f32, tag="sc", name="pk")
                    for et in range(NE):
                        nc.tensor.matmul(
                            pkl[:], wk_at(et), xT_sb[:, n, et],
                            start=(et == 0), stop=(et == NE - 1),
                        )
                    nc.vector.tensor_scalar_add(
                        out=kT[:, n * 512:(n + 1) * 512], in0=pkl[:],
                        scalar1=bk_sb,
                    )

                def q_proj_quanta(n, j):
                    """q-proj unit as 4 self-contained quanta (128 cols each)."""

                    def quarter(qi):
                        def run():
                            pq = ps.tile([P, 128], f32, tag="trpy", bufs=2, name="pq")
                            cl = qi * 128
                            for et in range(NE):
                                nc.tensor.matmul(
                                    pq[:], wq_at(et, j),
                                    xT_sb[:, n, et, cl:cl + 128],
                                    start=(et == 0), stop=(et == NE - 1),
                                )
                            nc.vector.tensor_scalar_add(
                                out=qT[:, j, n * 512 + cl:n * 512 + cl + 128],
                                in0=pq[:], scalar1=bq_sb[:, j:j + 1],
                            )
                        return run

                    return [quarter(qi) for qi in range(4)]

                def v_proj(tt):
                    pv = ps.tile([P, 128], f32, tag="trpy", bufs=2, name="pv")
                    c, sl = tt // 4, (tt % 4) * 128
                    for et in range(NE):
                        nc.tensor.matmul(
                            pv[:], xT_sb[:, c, et, sl:sl + 128], wv_at(et),
                            start=(et == 0), stop=(et == NE - 1),
                        )
                    for g in range(2):
                        nc.vector.tensor_add(
                            out=v_aug[g][:, tt, 0:64],
                            in0=pv[:, g * 64:(g + 1) * 64],
                            in1=bv_sb[:, g * 64:(g + 1) * 64],
                        )

                def alloc_exps():
                    return [
                        attn.tile([P, NT, NJ, SB], bf16, tag=f"exp{g}", name=f"exp{g}")
                        for g in range(2)
                    ]

                def scores_half(sb, exps, half, popper=None):
                    """One t-half of scores + exp for query block sb."""
                    ssl = slice(sb * SB, (sb + 1) * SB)
                    for grp in range(2):
                        poff = grp * 64
                        sc = ps.tile([P, 2, NJ, SB], f32, tag="sc", name="sc")
                        for q in range(2):
                            tt = 2 * half + q
                            nc.tensor.matmul(
                                sc[:, q],
                                kT[poff:poff + 64, tt * 128:(tt + 1) * 128],
                                qT[poff:poff + 64, :, ssl],
                                start=True, stop=True,
                            )
                        nc.scalar.activation(
                            out=exps[grp][:, 2 * half:2 * half + 2],
                            in_=sc[:], func=Exp, scale=float(SCALE),
                        )
                        if popper is not None:
                            popper()

                def av_chunk(pav, exps, grp, j):
                    """One (grp, head) accumulation group: full t contraction.
                    Groups must run one-at-a-time per PSUM bank."""
                    for tt in range(NT):
                        nc.tensor.matmul(
                            pav[grp][:, j],
                            exps[grp][:, tt, j],
                            v_aug[grp][:, tt],
                            start=(tt == 0), stop=(tt == NT - 1),
                            skip_group_check=True,
                        )

                def normalize_muls(pav):
                    """DVE-only: 1/denom + scale; frees the pav PSUM tiles."""
                    recs = []
                    for grp in range(2):
                        rec = misc.tile([P, NJ], f32, tag=f"rec{grp}", bufs=2, name="rec")
                        nc.vector.reciprocal(out=rec[:], in_=pav[grp][:, :, 64:65])
                        recs.append(rec)
                    aosbs = []
                    for j in range(NJ):
                        aosb = misc.tile([P, 2, 64], bf16, tag="aosb", bufs=8, name="aosb")
                        for grp in range(2):
                            nc.vector.tensor_scalar_mul(
                                out=aosb[:, grp], in0=pav[grp][:, j, 0:64],
                                scalar1=recs[grp][:, j:j + 1],
                            )
                        aosbs.append(aosb)
                    return aosbs

                def transpose_quantum(aosbs, aoTt):
                    for j in range(NJ):
                        tr = ps.tile([P, SB], bf16, tag="trpy", bufs=2, name="tr")
                        nc.tensor.transpose(tr[:], aosbs[j][:], ident[:])
                        nc.vector.tensor_copy(out=aoTt[:, j], in_=tr[:])

                def out_proj_unit(sb, aoTt, n):
                    py = ps.tile([P, 512], f32, tag="trpy", bufs=2, name="py")
                    for j in range(NJ):
                        nc.tensor.matmul(
                            py[:], aoTt[:, j],
                            wo_sb[:, j, n * 512:(n + 1) * 512],
                            start=(j == 0), stop=(j == NJ - 1),
                        )
                    ysb = misc.tile([P, 512], bf16, tag="ysb", bufs=3, name="ysb")
                    nc.vector.tensor_copy(out=ysb[:], in_=py[:])
                    eng = nc.sync if n % 2 == 0 else nc.gpsimd
                    eng.dma_start(out=y[:, sb, n * 512:(n + 1) * 512], in_=ysb[:])

                # ---- rolling work queue ----
                workq = []
                state = {"iter": -1, "h": 0}

                def eligible(ent):
                    if ent["hold_iter"] is not None and state["iter"] < ent["hold_iter"]:
                        return False
                    if ent["av_iter"] is not None:
                        if state["iter"] <= ent["av_iter"] and state["h"] < 1:
                            return False
                        return not any(e["before_av"] for e in workq)
                    return True

                def pop_quanta(budget, pred=None):
                    spent = 0
                    while workq and spent < budget:
                        idx = next(
                            (k for k, e in enumerate(workq)
                             if eligible(e) and (pred is None or pred(e))),
                            None,
                        )
                        if idx is None:
                            break
                        e = workq.pop(idx)
                        e["fn"]()
                        spent += e["cost"]

                def flush(pred):
                    for e in [e for e in workq if pred(e)]:
                        workq.remove(e)
                        e["fn"]()

                def push(cost, fn, av_iter=None, deadline=None, before_av=False,
                         hold_iter=None):
                    workq.append(
                        dict(cost=cost, fn=fn, av_iter=av_iter, deadline=deadline,
                             before_av=before_av, hold_iter=hold_iter)
                    )

                def q_block(b):
                    """quanta producing qT columns for scores block b."""
                    n, qi = b // 4, b % 4
                    return [q_proj_quanta(n, j)[qi] for j in range(NJ)]

                # ---- prologue: warmup covered k(0)/q(block 0)/v(0..3); the
                # remaining q(n=0) quanta and v units are fillers. Bridge the
                # DVE bias-adds with a few pops, then scores(0) with kT(1..3)
                # just-in-time. ----
                for b in range(1, 4):
                    for fn in q_block(b):
                        push(860, fn, deadline=b)
                for tt in range(4, NT):
                    push(880, (lambda tt=tt: v_proj(tt)), before_av=True)
                exps_cur = alloc_exps()
                # bridge the warmup bias-add chain with v units only (their
                # v_aug writes don't gate the first scores matmuls; q quanta
                # would add qT writes the sc matmuls wait on)
                pop_quanta(2600, pred=lambda e: e["before_av"])
                for h in range(8):
                    if h in (2, 4, 6):
                        k_proj(h // 2)
                    scores_half(0, exps_cur, h,
                                (lambda: pop_quanta(250)) if h < 2 else None)

                # ---- software-pipelined main loop ----
                ao_prev = None
                q_pushed = set()
                for i in range(NSB):
                    PHASE[0] = f"it{i:02d}"
                    state["iter"], state["h"] = i, 0
                    has_next = i + 1 < NSB
                    exps_next = alloc_exps() if has_next else None
                    ecur = exps_cur
                    if has_next:
                        pav = [
                            ps.tile([P, NJ, 65], f32, tag=f"pav{g}", bufs=1, name=f"pav{g}")
                            for g in range(2)
                        ]
                        for g in range(2):
                            for j in range(NJ):
                                push(
                                    450,
                                    (lambda g=g, j=j, pav=pav, e=ecur: av_chunk(pav, e, g, j)),
                                    av_iter=i,
                                )
                    if ao_prev is not None:
                        ao = ao_prev
                        last_op = None
                        for n in range(4):
                            fn = (lambda n=n, ao=ao, s=i - 1: out_proj_unit(s, ao, n))
                            if i == NSB - 1 and n == 3:
                                last_op = fn  # emitted between av2s and the nm chain
                            else:
                                push(860, fn, hold_iter=None)
                    b = i + 2
                    if 4 <= b <= NSB - 1:
                        for fn in q_block(b):
                            push(860, fn, deadline=b,
                                 hold_iter=(14 if b == 15 else None))

                    if has_next:
                        flush(lambda e: e["deadline"] is not None and e["deadline"] <= i + 1)
                        for h in range(8):
                            state["h"] = h
                            scores_half(i + 1, exps_next, h, lambda: pop_quanta(700))
                        state["h"] = 8
                        flush(lambda e: e["before_av"])
                        flush(lambda e: e["av_iter"] == i)
                        aosbs = normalize_muls(pav)
                        aoTt = attn.tile([P, NJ, SB], bf16, tag="aoT", bufs=2, name="aoTt")
                        push(300, (lambda a=aosbs, t=aoTt: transpose_quantum(a, t)))
                        ao_prev = aoTt
                        exps_cur = exps_next
                        continue

                    # ---- last iteration (i == 15) ----
                    # pav banks pre-zeroed by PE zeros-matmuls -> the 8 AV
                    # chains accumulate start-free, split into an ungated part
                    # (tt<12) after the out_proj(14) filler and a gated tail
                    # (tt>=12, waits on ACT's trailing exps).
                    PHASE[0] = "ep"
                    state["h"] = 8
                    # pav split into lo (j0,j1) / hi (j2,j3) tiles, each holding
                    # both groups in one bank: the nm chain for j0/j1 then only
                    # waits the lo AV writes, starting the transpose/out-proj
                    # pipeline two AV chains earlier.
                    pav_lo = ps.tile([P, 2, 2, 65], f32, tag="pav0", bufs=1, name="pavlo")
                    pav_hi = ps.tile([P, 2, 2, 65], f32, tag="pav1", bufs=1, name="pavhi")
                    for t_ in (pav_lo, pav_hi):
                        nc.tensor.matmul(
                            t_[:], zrow[0:1, 0:128], zrow[0:1, 0:260],
                            start=True, stop=True,
                        )

                    def pav_at(grp, j):
                        t_ = pav_lo if j < 2 else pav_hi
                        return t_[:, grp, j % 2]
                    # ungated filler: tr(14) + out_proj(14)
                    flush(lambda e: e["av_iter"] is None)

                    def av_part(grp, j, t0, t1):
                        for tt in range(t0, t1):
                            nc.tensor.matmul(
                                pav_at(grp, j),
                                ecur[grp][:, tt, j],
                                v_aug[grp][:, tt],
                                start=False, stop=(tt == NT - 1),
                                skip_group_check=True,
                            )

                    for j in range(NJ):
                        for g in range(2):
                            av_part(g, j, 0, 12)
                    for j in range(NJ):
                        for g in range(2):
                            av_part(g, j, 12, NT)
                    if last_op is not None:
                        # out_proj(14, n3): fills PE while DVE runs the nm
                        # chain; its drain rides ACT+Pool to keep DVE clear.
                        py14 = ps.tile([P, 512], f32, tag="sc", name="py14")
                        for j in range(NJ):
                            nc.tensor.matmul(
                                py14[:], ao_prev[:, j],
                                wo_sb[:, j, 3 * 512:4 * 512],
                                start=(j == 0), stop=(j == NJ - 1),
                            )
                        ysb14 = misc.tile([P, 512], bf16, tag="ysb", bufs=3, name="ysb")
                        nc.scalar.activation(out=ysb14[:], in_=py14[:], func=Copy)
                        nc.gpsimd.dma_start(out=y[:, NSB - 2, 3 * 512:4 * 512], in_=ysb14[:])
                    # per-head normalize (DVE reads pav after all AV writes ->
                    # no tile-WAR backpressure on PE)
                    aosbs = []
                    for j in range(NJ):
                        aosb = misc.tile([P, 2, 64], bf16, tag="aosb", bufs=8, name="aosbl")
                        for g in range(2):
                            rec = misc.tile([P, 1], f32, tag="recl", bufs=4, name="recl")
                            nc.vector.reciprocal(out=rec[:], in_=pav_at(g, j)[:, 64:65])
                            nc.vector.tensor_scalar_mul(
                                out=aosb[:, g], in0=pav_at(g, j)[:, 0:64],
                                scalar1=rec[:, 0:1],
                            )
                        aosbs.append(aosb)
                    # transpose + out-proj pieces, interleaved so PE never sits
                    # behind a DVE copy in program order.
                    # four independent piece tiles (two sc slots + the pav
                    # slots, which free right as the pieces start) -> four
                    # parallel one-copy drain chains.
                    pyA = ps.tile([P, 512], f32, tag="sc", name="pyA")
                    pyB = ps.tile([P, 512], f32, tag="sc", name="pyB")
                    pyC = ps.tile([P, 512], f32, tag="pav0", bufs=1, name="pyC")
                    pyD = ps.tile([P, 512], f32, tag="pav1", bufs=1, name="pyD")
                    aoT15 = attn.tile([P, NJ, SB], bf16, tag="aoT", bufs=2, name="aoT15")
                    # B last: its PSUM slot frees only after out_proj(14,n3)
                    # drains, so its j0 matmul must trail the others.
                    pieces = [
                        ("A", lambda: pyA[:], 0, 512, "j0"),
                        ("C", lambda: pyC[:], 1024, 512, "j0"),
                        ("D", lambda: pyD[:], 1536, 512, "j0"),
                        ("B", lambda: pyB[:], 512, 512, "j0"),
                    ]
                    trs = []

                    def tr_j(j):
                        tr = ps.tile([P, SB], bf16, tag="trpy", bufs=2, name="tr")
                        nc.tensor.transpose(tr[:], aosbs[j][:], ident[:])
                        trs.append(tr)

                    def copy_j(j):
                        nc.vector.tensor_copy(out=aoT15[:, j], in_=trs[j][:])

                    def pieces_j(j):
                        for _, ap, col, w, st in pieces:
                            nc.tensor.matmul(
                                ap(), aoT15[:, j],
                                wo_sb[:, j, col:col + w],
                                start=(st == "j0" and j == 0), stop=(j == NJ - 1),
                                skip_group_check=True,
                            )

                    tr_j(0)
                    tr_j(1)
                    copy_j(0)
                    pieces_j(0)
                    tr_j(2)
                    copy_j(1)
                    pieces_j(1)
                    tr_j(3)
                    copy_j(2)
                    pieces_j(2)
                    copy_j(3)
                    pieces_j(3)
                    # drain: copies spread over DVE/ACT/Pool, DMAs over
                    # sync/scalar/gpsimd; all piece matmuls precede all copies,
                    # so no WAR dependency can stall the PE.
                    # one copy per tile -> four parallel chains on
                    # DVE/Pool/ACT/Pool with DMAs spread over all queues.
                    # GPSIMD cannot read PSUM on real HW: copies go on
                    # DVE/ACT only, alternating so each engine's chain is short.
                    cp_eng = [
                        nc.vector.tensor_copy,        # A
                        None,                         # C -> ACT Copy
                        nc.vector.tensor_copy,        # D
                        None,                         # B -> ACT Copy
                    ]
                    dma_eng = [nc.sync, nc.gpsimd, nc.sync, nc.scalar]
                    for pi, (_, ap, col, w, _st) in enumerate(pieces):
                        ysb = misc.tile([P, w], bf16, tag="ydr", bufs=5, name="ydr")
                        if cp_eng[pi] is None:
                            nc.scalar.activation(out=ysb[:], in_=ap(), func=Copy)
                        else:
                            cp_eng[pi](out=ysb[:], in_=ap())
                        dma_eng[pi].dma_start(
                            out=y[:, NSB - 1, col:col + w], in_=ysb[:]
                        )
                flush(lambda e: True)
    nc.compile()
    return nc


def _get_program():
    global _compiled
    if _compiled is None:
        _compiled = build_gqa_program()
    return _compiled


def _wrap_pmn(a2d, ntile):
    """[R, C] -> [128, R/128, C] with row r at (r % 128, r // 128)."""
    r, c = a2d.shape
    return np.ascontiguousarray(a2d.reshape(ntile, P, c).transpose(1, 0, 2))


def shard_inputs(x, Wq, bq, Wk, bk, Wv, bv, Wo):
    """Build the 8 per-core input maps (host-side shard + transpose + cast)."""
    ins = []
    for c in range(8):
        b, g = c // 4, c % 4
        # q-head columns for this core, ordered (j, pair, d):
        # global q-col = (2g + pair) * 256 + j * 64 + d
        j_idx, pair_idx, d_idx = np.meshgrid(
            np.arange(NJ), np.arange(2), np.arange(64), indexing="ij"
        )
        qcols = ((2 * g + pair_idx) * (GROUP * 64) + j_idx * 64 + d_idx).reshape(-1)
        kvcols = np.arange(g * 128, (g + 1) * 128)  # kv heads 2g, 2g+1

        xT = np.ascontiguousarray(x[b].T)  # [E, S] f32
        xTw = _wrap_pmn(xT, NE)  # [128, NE, S]
        xTc = np.ascontiguousarray(
            xTw.reshape(P, NE, NCH, 512).transpose(0, 2, 1, 3)
        )  # [128, NCH, NE, 512]
        wk_w = _wrap_pmn(Wk[:, kvcols], NE)  # [128, NE, 128]
        wv_w = _wrap_pmn(Wv[:, kvcols], NE)  # [128, NE, 128]
        wq_w = _wrap_pmn(Wq[:, qcols], NE)  # [128, NE, 512]
        wmix = np.concatenate([wk_w, wv_w, wq_w], axis=2)  # [128, NE, 768]
        bqd = np.ascontiguousarray(bq[qcols].reshape(NJ, P).T.astype(np.float32))
        bkd = bk[kvcols].reshape(P, 1).astype(np.float32)
        bvd = np.ascontiguousarray(
            np.broadcast_to(bv[kvcols][None, :], (P, 128))
        ).astype(np.float32)
        ball = np.concatenate([bqd, bkd, bvd], axis=1).astype(np.float32)
        ins.append(
            {
                "xT": xTc.astype(BF16),
                "wmix": np.ascontiguousarray(wmix).astype(BF16),
                "wo": _wrap_pmn(Wo[qcols, :], NJ).astype(BF16),
                "ball": ball,
            }
        )
    return ins


def gather_outputs(results, bo):
    """Sum the 4 row-parallel partials per batch, add bias."""
    y = np.zeros((B, S, E), np.float64)
    for c in range(8):
        b = c // 4
        part = results[c]["y"]  # [128, NT, E] bf16
        y[b] += part.transpose(1, 0, 2).reshape(S, E).astype(np.float64)
    return (y + bo.astype(np.float64)).astype(np.float32)


def kernel(x, Wq, bq, Wk, bk, Wv, bv, Wo, bo):
    from concourse.bass_utils import run_bass_kernel_spmd

    x = np.asarray(x, np.float32)
    nc = _get_program()
    ins = shard_inputs(
        x,
        np.asarray(Wq, np.float32),
        np.asarray(bq, np.float32),
        np.asarray(Wk, np.float32),
        np.asarray(bk, np.float32),
        np.asarray(Wv, np.float32),
        np.asarray(bv, np.float32),
        np.asarray(Wo, np.float32),
    )
    r = run_bass_kernel_spmd(nc, ins, list(range(8)), **_RUN_KWARGS)
    globals()["_last_run"] = r
    return gather_outputs(r.results, np.asarray(bo, np.float32))


# revision 45
# speedup vs baseline: 1.0015x; 1.0015x over previous
"""Grouped-Query Attention (B=2, S=2048, E=2048, 32 q heads, 8 kv heads, d=64)
on 8 Trainium2 NeuronCores.

Sharding: 8 cores = 2 batches x 4 kv-head-groups. Each core handles one batch
and 2 kv heads (= 8 q heads), computing its slice of attention plus the
row-parallel partial out-projection. The host sums the 4 partial outputs per
batch (no on-device collectives needed) and adds the output bias.

On-device pipeline per core (matmuls bf16, fp32 accumulation):
  warmup: k(0) + v(tt 0..3) + q(n=0, qi=0) et-interleaved against et-granular
  DMA (xT chunk0 on sync, fused weights on scalar) so PE is fed from ~2us ->
  scoresT = k @ qT (t on partitions) -> exp on ScalarE (scale 1/8 folded in,
  no max-subtraction: scores are O(5) for unit-variance inputs) ->
  AV^T: pav[q, d|denom] += exps_tile.T @ [v | ones]  (the ones column gives
  the softmax denominator directly, per (q, head), no cross-partition
  reduction needed) -> reciprocal + normalize (DVE) -> PE transpose back to
  [d, q] -> out-projection, software-pipelined one block behind attention.
  Last iteration: pav banks pre-zeroed by PE zeros-matmuls so the 8 AV
  chains run start-free split into an ungated (tt<12) and a gated (tt>=12)
  part around the out_proj(14) filler; per-head normalize/transpose/out-proj
  pipeline ordered to avoid tile-granular WAR serialization; final output
  pieces drain on DVE+ACT copies and sync+scalar DMA queues.
"""

import sys

sys.path.insert(0, "/opt/trn_rl_repo")

import numpy as np
import ml_dtypes

BF16 = ml_dtypes.bfloat16

P = 128
B, S, E = 2, 2048, 2048
NUM_HEADS, NUM_KV_HEADS, HEAD_DIM = 32, 8, 64
GROUP = NUM_HEADS // NUM_KV_HEADS  # 4
NE = E // P  # 16 e-tiles (contraction tiles for projections)
NT = S // P  # 16 t-tiles (key/value positions)
NJ = GROUP  # 4 q-heads per kv head
SB = 128  # query-block size
NSB = S // SB  # 16 query blocks
NCH = 4  # xT S-chunks
SCALE = 1.0 / np.sqrt(HEAD_DIM)

_compiled = None  # cached program
_RUN_KWARGS = {}  # test harness may set e.g. {"trace": True}
_last_run = None  # BassKernelResults of the most recent kernel() call
PHASE = [""]  # build-time phase label, for the analysis tooling


def build_gqa_program():
    from concourse import bacc, mybir, tile, masks

    f32 = mybir.dt.float32
    bf16 = mybir.dt.bfloat16
    Exp = mybir.ActivationFunctionType.Exp
    Copy = mybir.ActivationFunctionType.Copy

    nc = bacc.Bacc(None, target_bir_lowering=False, debug=False)
    with tile.TileContext(nc) as tc:
        with tc.tile_pool(name="dram", bufs=1, space="DRAM") as dram:
            xT = dram.tile([P, NCH, NE, 512], bf16, kind="ExternalInput", name="xT", uniquify=False)
            # wmix[:, et] = [wk 128 | wv 128 | wq j0..j3 512] for that e-tile
            wmix = dram.tile([P, NE, 768], bf16, kind="ExternalInput", name="wmix", uniquify=False)
            wo = dram.tile([P, NJ, E], bf16, kind="ExternalInput", name="wo", uniquify=False)
            # ball = [bq j0..j3 (per-partition) | bk (per-partition) | bv 128 (free-dim)]
            ball = dram.tile([P, 133], f32, kind="ExternalInput", name="ball", uniquify=False)
            y = dram.tile([P, NT, E], bf16, kind="ExternalOutput", name="y", uniquify=False)

            with (
                tc.tile_pool(name="win", bufs=1) as win,
                tc.tile_pool(name="proj", bufs=1) as proj,
                tc.tile_pool(name="attn", bufs=2) as attn,
                tc.tile_pool(name="misc", bufs=2) as misc,
                tc.tile_pool(name="ps", bufs=2, space="PSUM") as ps,
            ):
                # ---- input DMAs, 3 queues, consumption-ordered. Warmup data
                # (chunk0 + wmix) is split across all three queues; the exp
                # table load pins the scalar queue's head, so the first
                # critical pieces ride sync/gpsimd.
                xT_sb = win.tile([P, NCH, NE, 512], bf16)
                wmix_sb = win.tile([P, NE, 768], bf16)
                ball_sb = win.tile([P, 133], f32)
                wo_sb = win.tile([P, NJ, E], bf16)

                nc.sync.dma_start(out=wmix_sb[:, 0, 0:256], in_=wmix[:, 0, 0:256])
                nc.sync.dma_start(out=xT_sb[:, 0, 1:2], in_=xT[:, 0, 1:2])
                nc.sync.dma_start(out=xT_sb[:, 0, 2:4], in_=xT[:, 0, 2:4])
                nc.sync.dma_start(out=xT_sb[:, 0, 4:6], in_=xT[:, 0, 4:6])
                nc.sync.dma_start(out=xT_sb[:, 0, 6:8], in_=xT[:, 0, 6:8])
                nc.sync.dma_start(out=wmix_sb[:, 11:16], in_=wmix[:, 11:16])
                nc.sync.dma_start(out=ball_sb[:], in_=ball[:])
                nc.sync.dma_start(out=xT_sb[:, 2, 0:8], in_=xT[:, 2, 0:8])
                nc.sync.dma_start(out=xT_sb[:, 2, 8:16], in_=xT[:, 2, 8:16])

                nc.scalar.dma_start(out=wmix_sb[:, 0, 256:768], in_=wmix[:, 0, 256:768])
                nc.scalar.dma_start(out=wmix_sb[:, 1:2], in_=wmix[:, 1:2])
                nc.scalar.dma_start(out=wmix_sb[:, 2:4], in_=wmix[:, 2:4])
                nc.scalar.dma_start(out=wmix_sb[:, 7:11], in_=wmix[:, 7:11])
                nc.scalar.dma_start(out=xT_sb[:, 3, 0:8], in_=xT[:, 3, 0:8])
                nc.scalar.dma_start(out=xT_sb[:, 3, 8:16], in_=xT[:, 3, 8:16])

                nc.gpsimd.dma_start(out=xT_sb[:, 0, 0:1], in_=xT[:, 0, 0:1])
                nc.gpsimd.dma_start(out=xT_sb[:, 0, 8:12], in_=xT[:, 0, 8:12])
                nc.gpsimd.dma_start(out=wmix_sb[:, 4:7], in_=wmix[:, 4:7])
                nc.gpsimd.dma_start(out=xT_sb[:, 0, 12:16], in_=xT[:, 0, 12:16])
                # v with a ones column per kv head: [v_g | 1] -> denominator
                # comes out of the AV matmul for free.
                zrow = win.tile([1, 512], bf16)
                nc.gpsimd.memset(zrow[:], 0.0)
                v_aug = [win.tile([P, NT, 65], bf16, name=f"vaug{g}") for g in range(2)]
                for g in range(2):
                    nc.gpsimd.memset(v_aug[g][:, :, 64:65], 1.0)
                ident = win.tile([P, P], bf16)
                masks.make_identity(nc, ident[:])
                nc.gpsimd.dma_start(out=xT_sb[:, 1, 0:8], in_=xT[:, 1, 0:8])
                nc.gpsimd.dma_start(out=xT_sb[:, 1, 8:16], in_=xT[:, 1, 8:16])
                nc.gpsimd.dma_start(out=wo_sb[:], in_=wo[:])

                def wk_at(et):
                    return wmix_sb[:, et, 0:128]

                def wv_at(et):
                    return wmix_sb[:, et, 128:256]

                def wq_at(et, j):
                    return wmix_sb[:, et, 256 + j * 128:256 + (j + 1) * 128]

                bq_sb = ball_sb[:, 0:NJ]
                bk_sb = ball_sb[:, NJ:NJ + 1]
                bv_sb = ball_sb[:, NJ + 1:NJ + 1 + 128]

                # exp table warm-up; reads wmix so the ACT table load schedules
                # after the warmup-critical DMAs on the scalar queue.
                warm = misc.tile([1, 1], f32, tag="warm")
                nc.scalar.activation(out=warm[:], in_=wmix_sb[0:1, 15, 0:1], func=Exp)

                kT = proj.tile([P, S], bf16)
                qT = proj.tile([P, NJ, S], bf16)

                # ---- warmup: k(0) + v(tt 0..3) + q(n=0, qi=0) et-interleaved.
                # pvpack/pqpack hold 4 accumulation quadrants per bank; the
                # very first matmul's start=True lazily zeroes the whole bank,
                # everything else accumulates start-free.
                PHASE[0] = "warm"
                pk = ps.tile([P, 512], f32, tag="sc", name="pk")
                pvpack = ps.tile([P, 4, 128], f32, tag="sc", name="pvpack")
                pqpack = ps.tile([P, 4, 128], f32, tag="pav0", bufs=1, name="pqpack")

                def warm_q(qet):
                    for j in range(NJ):
                        nc.tensor.matmul(
                            pqpack[:, j], wq_at(qet, j), xT_sb[:, 0, qet, 0:128],
                            start=(qet == 0 and j == 0), stop=(qet == NE - 1),
                            skip_group_check=True,
                        )

                # q lags k/v by two e-tiles: its weights ride the scalar queue
                # behind the exp-table load, and the lag keeps the in-order PE
                # stream from blocking on them.
                for et in range(NE):
                    if et >= 3:
                        warm_q(et - 3)
                    nc.tensor.matmul(
                        pk[:], wk_at(et), xT_sb[:, 0, et],
                        start=(et == 0), stop=(et == NE - 1),
                    )
                    for tt in range(4):
                        nc.tensor.matmul(
                            pvpack[:, tt],
                            xT_sb[:, 0, et, tt * 128:(tt + 1) * 128],
                            wv_at(et),
                            start=(et == 0 and tt == 0), stop=(et == NE - 1),
                            skip_group_check=True,
                        )
                for qet in range(NE - 3, NE):
                    warm_q(qet)
                # pops: the first scores matmul gates on kT+qT; split the
                # bias-adds across DVE and ACT (Copy with per-partition bias)
                # so the gate chain runs on two engines in parallel.
                nc.vector.tensor_scalar_add(
                    out=kT[:, 0:512], in0=pk[:], scalar1=bk_sb
                )
                for j in (0, 1):
                    nc.vector.tensor_scalar_add(
                        out=qT[:, j, 0:128], in0=pqpack[:, j],
                        scalar1=bq_sb[:, j:j + 1],
                    )
                Ident = mybir.ActivationFunctionType.Identity
                for j in (2, 3):
                    nc.scalar.activation(
                        out=qT[:, j, 0:128], in_=pqpack[:, j], func=Ident,
                        bias=bq_sb[:, j:j + 1],
                    )
                for tt in range(4):
                    for g in range(2):
                        nc.vector.tensor_add(
                            out=v_aug[g][:, tt, 0:64],
                            in0=pvpack[:, tt, g * 64:(g + 1) * 64],
                            in1=bv_sb[:, g * 64:(g + 1) * 64],
                        )

                # ---- work units ----
                def k_proj(n):
                    pkl = ps.tile([P, 512], f32, tag="sc", name="pk")
                    for et in range(NE):
                        nc.tensor.matmul(
                            pkl[:], wk_at(et), xT_sb[:, n, et],
                            start=(et == 0), stop=(et == NE - 1),
                        )
                    nc.vector.tensor_scalar_add(
                        out=kT[:, n * 512:(n + 1) * 512], in0=pkl[:],
                        scalar1=bk_sb,
                    )

                def q_proj_quanta(n, j):
                    """q-proj unit as 4 self-contained quanta (128 cols each)."""

                    def quarter(qi):
                        def run():
                            pq = ps.tile([P, 128], f32, tag="trpy", bufs=2, name="pq")
                            cl = qi * 128
                            for et in range(NE):
                                nc.tensor.matmul(
                                    pq[:], wq_at(et, j),
                                    xT_sb[:, n, et, cl:cl + 128],
                                    start=(et == 0), stop=(et == NE - 1),
                                )
                            nc.vector.tensor_scalar_add(
                                out=qT[:, j, n * 512 + cl:n * 512 + cl + 128],
                                in0=pq[:], scalar1=bq_sb[:, j:j + 1],
                            )
                        return run

                    return [quarter(qi) for qi in range(4)]

                def v_proj(tt):
                    pv = ps.tile([P, 128], f32, tag="trpy", bufs=2, name="pv")
                    c, sl = tt // 4, (tt % 4) * 128
                    for et in range(NE):
                        nc.tensor.matmul(
                            pv[:], xT_sb[:, c, et, sl:sl + 128], wv_at(et),
                            start=(et == 0), stop=(et == NE - 1),
                        )
                    for g in range(2):
                        nc.vector.tensor_add(
                            out=v_aug[g][:, tt, 0:64],
                            in0=pv[:, g * 64:(g + 1) * 64],
                            in1=bv_sb[:, g * 64:(g + 1) * 64],
                        )

                def alloc_exps():
                    return [
                        attn.tile([P, NT, NJ, SB], bf16, tag=f"exp{g}", name=f"exp{g}")
                        for g in range(2)
                    ]

                def scores_half(sb, exps, half, popper=None):
                    """One t-half of scores + exp for query block sb."""
                    ssl = slice(sb * SB, (sb + 1) * SB)
                    for grp in range(2):
                        poff = grp * 64
                        sc = ps.tile([P, 2, NJ, SB], f32, tag="sc", name="sc")
                        for q in range(2):
                            tt = 2 * half + q
                            nc.tensor.matmul(
                                sc[:, q],
                                kT[poff:poff + 64, tt * 128:(tt + 1) * 128],
                                qT[poff:poff + 64, :, ssl],
                                start=True, stop=True,
                            )
                        nc.scalar.activation(
                            out=exps[grp][:, 2 * half:2 * half + 2],
                            in_=sc[:], func=Exp, scale=float(SCALE),
                        )
                        if popper is not None:
                            popper()

                def av_chunk(pav, exps, grp, j):
                    """One (grp, head) accumulation group: full t contraction.
                    Groups must run one-at-a-time per PSUM bank."""
                    for tt in range(NT):
                        nc.tensor.matmul(
                            pav[grp][:, j],
                            exps[grp][:, tt, j],
                            v_aug[grp][:, tt],
                            start=(tt == 0), stop=(tt == NT - 1),
                            skip_group_check=True,
                        )

                def normalize_muls(pav):
                    """DVE-only: 1/denom + scale; frees the pav PSUM tiles."""
                    recs = []
                    for grp in range(2):
                        rec = misc.tile([P, NJ], f32, tag=f"rec{grp}", bufs=2, name="rec")
                        nc.vector.reciprocal(out=rec[:], in_=pav[grp][:, :, 64:65])
                        recs.append(rec)
                    aosbs = []
                    for j in range(NJ):
                        aosb = misc.tile([P, 2, 64], bf16, tag="aosb", bufs=8, name="aosb")
                        for grp in range(2):
                            nc.vector.tensor_scalar_mul(
                                out=aosb[:, grp], in0=pav[grp][:, j, 0:64],
                                scalar1=recs[grp][:, j:j + 1],
                            )
                        aosbs.append(aosb)
                    return aosbs

                def transpose_quantum(aosbs, aoTt):
                    for j in range(NJ):
                        tr = ps.tile([P, SB], bf16, tag="trpy", bufs=2, name="tr")
                        nc.tensor.transpose(tr[:], aosbs[j][:], ident[:])
                        nc.vector.tensor_copy(out=aoTt[:, j], in_=tr[:])

                def out_proj_unit(sb, aoTt, n):
                    py = ps.tile([P, 512], f32, tag="trpy", bufs=2, name="py")
                    for j in range(NJ):
                        nc.tensor.matmul(
                            py[:], aoTt[:, j],
                            wo_sb[:, j, n * 512:(n + 1) * 512],
                            start=(j == 0), stop=(j == NJ - 1),
                        )
                    ysb = misc.tile([P, 512], bf16, tag="ysb", bufs=3, name="ysb")
                    nc.vector.tensor_copy(out=ysb[:], in_=py[:])
                    eng = nc.sync if n % 2 == 0 else nc.gpsimd
                    eng.dma_start(out=y[:, sb, n * 512:(n + 1) * 512], in_=ysb[:])

                # ---- rolling work queue ----
                workq = []
                state = {"iter": -1, "h": 0}

                def eligible(ent):
                    if ent["hold_iter"] is not None and state["iter"] < ent["hold_iter"]:
                        return False
                    if ent["av_iter"] is not None:
                        if state["iter"] <= ent["av_iter"] and state["h"] < 1:
                            return False
                        return not any(e["before_av"] for e in workq)
                    return True

                def pop_quanta(budget, pred=None):
                    spent = 0
                    while workq and spent < budget:
                        idx = next(
                            (k for k, e in enumerate(workq)
                             if eligible(e) and (pred is None or pred(e))),
                            None,
                        )
                        if idx is None:
                            break
                        e = workq.pop(idx)
                        e["fn"]()
                        spent += e["cost"]

                def flush(pred):
                    for e in [e for e in workq if pred(e)]:
                        workq.remove(e)
                        e["fn"]()

                def push(cost, fn, av_iter=None, deadline=None, before_av=False,
                         hold_iter=None):
                    workq.append(
                        dict(cost=cost, fn=fn, av_iter=av_iter, deadline=deadline,
                             before_av=before_av, hold_iter=hold_iter)
                    )

                def q_block(b):
                    """quanta producing qT columns for scores block b."""
                    n, qi = b // 4, b % 4
                    return [q_proj_quanta(n, j)[qi] for j in range(NJ)]

                # ---- prologue: warmup covered k(0)/q(block 0)/v(0..3); the
                # remaining q(n=0) quanta and v units are fillers. Bridge the
                # DVE bias-adds with a few pops, then scores(0) with kT(1..3)
                # just-in-time. ----
                for b in range(1, 4):
                    for fn in q_block(b):
                        push(860, fn, deadline=b)
                for tt in range(4, NT):
                    push(880, (lambda tt=tt: v_proj(tt)), before_av=True)
                exps_cur = alloc_exps()
                # bridge the warmup bias-add chain with v units only (their
                # v_aug writes don't gate the first scores matmuls; q quanta
                # would add qT writes the sc matmuls wait on)
                pop_quanta(2600, pred=lambda e: e["before_av"])
                for h in range(8):
                    if h in (2, 4, 6):
                        k_proj(h // 2)
                    scores_half(0, exps_cur, h,
                                (lambda: pop_quanta(250)) if h < 2 else None)

                # ---- software-pipelined main loop ----
                ao_prev = None
                q_pushed = set()
                for i in range(NSB):
                    PHASE[0] = f"it{i:02d}"
                    state["iter"], state["h"] = i, 0
                    has_next = i + 1 < NSB
                    exps_next = alloc_exps() if has_next else None
                    ecur = exps_cur
                    if has_next:
                        pav = [
                            ps.tile([P, NJ, 65], f32, tag=f"pav{g}", bufs=1, name=f"pav{g}")
                            for g in range(2)
                        ]
                        for g in range(2):
                            for j in range(NJ):
                                push(
                                    450,
                                    (lambda g=g, j=j, pav=pav, e=ecur: av_chunk(pav, e, g, j)),
                                    av_iter=i,
                                )
                    if ao_prev is not None:
                        ao = ao_prev
                        last_op = None
                        for n in range(4):
                            fn = (lambda n=n, ao=ao, s=i - 1: out_proj_unit(s, ao, n))
                            if i == NSB - 1 and n == 3:
                                last_op = fn  # emitted between av2s and the nm chain
                            else:
                                push(860, fn, hold_iter=None)
                    b = i + 2
                    if 4 <= b <= NSB - 1:
                        for fn in q_block(b):
                            push(860, fn, deadline=b,
                                 hold_iter=(14 if b == 15 else None))

                    if has_next:
                        flush(lambda e: e["deadline"] is not None and e["deadline"] <= i + 1)
                        for h in range(8):
                            state["h"] = h
                            scores_half(i + 1, exps_next, h, lambda: pop_quanta(700))
                        state["h"] = 8
                        flush(lambda e: e["before_av"])
                        flush(lambda e: e["av_iter"] == i)
                        aosbs = normalize_muls(pav)
                        aoTt = attn.tile([P, NJ, SB], bf16, tag="aoT", bufs=2, name="aoTt")
                        push(300, (lambda a=aosbs, t=aoTt: transpose_quantum(a, t)))
                        ao_prev = aoTt
                        exps_cur = exps_next
                        continue

                    # ---- last iteration (i == 15) ----
                    # pav banks pre-zeroed by PE zeros-matmuls -> the 8 AV
                    # chains accumulate start-free, split into an ungated part
                    # (tt<12) after the out_proj(14) filler and a gated tail
                    # (tt>=12, waits on ACT's trailing exps).
                    PHASE[0] = "ep"
                    state["h"] = 8
                    # pav split into lo (j0,j1) / hi (j2,j3) tiles, each holding
                    # both groups in one bank: the nm chain for j0/j1 then only
                    # waits the lo AV writes, starting the transpose/out-proj
                    # pipeline two AV chains earlier.
                    pav_lo = ps.tile([P, 2, 2, 65], f32, tag="pav0", bufs=1, name="pavlo")
                    pav_hi = ps.tile([P, 2, 2, 65], f32, tag="pav1", bufs=1, name="pavhi")
                    for t_ in (pav_lo, pav_hi):
                        nc.tensor.matmul(
                            t_[:], zrow[0:1, 0:128], zrow[0:1, 0:260],
                            start=True, stop=True,
                        )

                    def pav_at(grp, j):
                        t_ = pav_lo if j < 2 else pav_hi
                        return t_[:, grp, j % 2]
                    # ungated filler: tr(14) + out_proj(14)
                    flush(lambda e: e["av_iter"] is None)

                    def av_part(grp, j, t0, t1):
                        for tt in range(t0, t1):
                            nc.tensor.matmul(
                                pav_at(grp, j),
                                ecur[grp][:, tt, j],
                                v_aug[grp][:, tt],
                                start=False, stop=(tt == NT - 1),
                                skip_group_check=True,
                            )

                    for j in range(NJ):
                        for g in range(2):
                            av_part(g, j, 0, 12)
                    for j in range(NJ):
                        for g in range(2):
                            av_part(g, j, 12, NT)
                    if last_op is not None:
                        # out_proj(14, n3): fills PE while DVE runs the nm
                        # chain; its drain rides ACT+Pool to keep DVE clear.
                        py14 = ps.tile([P, 512], f32, tag="sc", name="py14")
                        for j in range(NJ):
                            nc.tensor.matmul(
                                py14[:], ao_prev[:, j],
                                wo_sb[:, j, 3 * 512:4 * 512],
                                start=(j == 0), stop=(j == NJ - 1),
                            )
                        ysb14 = misc.tile([P, 512], bf16, tag="ysb", bufs=3, name="ysb")
                        nc.scalar.activation(out=ysb14[:], in_=py14[:], func=Copy)
                        nc.gpsimd.dma_start(out=y[:, NSB - 2, 3 * 512:4 * 512], in_=ysb14[:])
                    # per-head normalize (DVE reads pav after all AV writes ->
                    # no tile-WAR backpressure on PE)
                    aosbs = []
                    for j in range(NJ):
                        aosb = misc.tile([P, 2, 64], bf16, tag="aosb", bufs=8, name="aosbl")
                        for g in range(2):
                            rec = misc.tile([P, 1], f32, tag="recl", bufs=4, name="recl")
                            nc.vector.reciprocal(out=rec[:], in_=pav_at(g, j)[:, 64:65])
                            nc.vector.tensor_scalar_mul(
                                out=aosb[:, g], in0=pav_at(g, j)[:, 0:64],
                                scalar1=rec[:, 0:1],
                            )
                        aosbs.append(aosb)
                    # transpose + out-proj pieces, interleaved so PE never sits
                    # behind a DVE copy in program order.
                    # four independent piece tiles (two sc slots + the pav
                    # slots, which free right as the pieces start) -> four
                    # parallel one-copy drain chains.
                    pyA = ps.tile([P, 512], f32, tag="sc", name="pyA")
                    pyB = ps.tile([P, 512], f32, tag="sc", name="pyB")
                    pyC = ps.tile([P, 512], f32, tag="pav0", bufs=1, name="pyC")
                    pyD = ps.tile([P, 512], f32, tag="pav1", bufs=1, name="pyD")
                    aoT15 = attn.tile([P, NJ, SB], bf16, tag="aoT", bufs=2, name="aoT15")
                    # B last: its PSUM slot frees only after out_proj(14,n3)
                    # drains, so its j0 matmul must trail the others.
                    pieces = [
                        ("A", lambda: pyA[:], 0, 512, "j0"),
                        ("C", lambda: pyC[:], 1024, 512, "j0"),
                        ("D", lambda: pyD[:], 1536, 512, "j0"),
                        ("B", lambda: pyB[:], 512, 512, "j0"),
                    ]
                    trs = []

                    def tr_j(j):
                        tr = ps.tile([P, SB], bf16, tag="trpy", bufs=2, name="tr")
                        nc.tensor.transpose(tr[:], aosbs[j][:], ident[:])
                        trs.append(tr)

                    def copy_j(j):
                        nc.vector.tensor_copy(out=aoT15[:, j], in_=trs[j][:])

                    def pieces_j(j):
                        for _, ap, col, w, st in pieces:
                            nc.tensor.matmul(
                                ap(), aoT15[:, j],
                                wo_sb[:, j, col:col + w],
                                start=(st == "j0" and j == 0), stop=(j == NJ - 1),
                                skip_group_check=True,
                            )

                    tr_j(0)
                    tr_j(1)
                    copy_j(0)
                    pieces_j(0)
                    tr_j(2)
                    copy_j(1)
                    pieces_j(1)
                    tr_j(3)
                    copy_j(2)
                    pieces_j(2)
                    copy_j(3)
                    pieces_j(3)
                    # drain: copies spread over DVE/ACT/Pool, DMAs over
                    # sync/scalar/gpsimd; all piece matmuls precede all copies,
                    # so no WAR dependency can stall the PE.
                    # one copy per tile -> four parallel chains on
                    # DVE/Pool/ACT/Pool with DMAs spread over all queues.
                    # GPSIMD cannot read PSUM on real HW: copies go on
                    # DVE/ACT only, alternating so each engine's chain is short.
                    cp_eng = [
                        nc.vector.tensor_copy,        # A
                        None,                         # C -> ACT Copy
                        nc.vector.tensor_copy,        # D
                        None,                         # B -> ACT Copy
                    ]
                    dma_eng = [nc.sync, nc.gpsimd, nc.sync, nc.scalar]
                    for pi, (_, ap, col, w, _st) in enumerate(pieces):
                        ysb = misc.tile([P, w], bf16, tag="ydr", bufs=5, name="ydr")
                        if cp_eng[pi] is None:
                            nc.scalar.activation(out=ysb[:], in_=ap(), func=Copy)
                        else:
                            cp_eng[pi](out=ysb[:], in_=ap())
                        dma_eng[pi].dma_start(
                            out=y[:, NSB - 1, col:col + w], in_=ysb[:]
                        )
                flush(lambda e: True)
    nc.compile()
    return nc


def _get_program():
    global _compiled
    if _compiled is None:
        _compiled = build_gqa_program()
    return _compiled


def _wrap_pmn(a2d, ntile):
    """[R, C] -> [128, R/128, C] with row r at (r % 128, r // 128)."""
    r, c = a2d.shape
    return np.ascontiguousarray(a2d.reshape(ntile, P, c).transpose(1, 0, 2))


def shard_inputs(x, Wq, bq, Wk, bk, Wv, bv, Wo):
    """Build the 8 per-core input maps (host-side shard + transpose + cast)."""
    ins = []
    for c in range(8):
        b, g = c // 4, c % 4
        # q-head columns for this core, ordered (j, pair, d):
        # global q-col = (2g + pair) * 256 + j * 64 + d
        j_idx, pair_idx, d_idx = np.meshgrid(
            np.arange(NJ), np.arange(2), np.arange(64), indexing="ij"
        )
        qcols = ((2 * g + pair_idx) * (GROUP * 64) + j_idx * 64 + d_idx).reshape(-1)
        kvcols = np.arange(g * 128, (g + 1) * 128)  # kv heads 2g, 2g+1

        xT = np.ascontiguousarray(x[b].T)  # [E, S] f32
        xTw = _wrap_pmn(xT, NE)  # [128, NE, S]
        xTc = np.ascontiguousarray(
            xTw.reshape(P, NE, NCH, 512).transpose(0, 2, 1, 3)
        )  # [128, NCH, NE, 512]
        wk_w = _wrap_pmn(Wk[:, kvcols], NE)  # [128, NE, 128]
        wv_w = _wrap_pmn(Wv[:, kvcols], NE)  # [128, NE, 128]
        wq_w = _wrap_pmn(Wq[:, qcols], NE)  # [128, NE, 512]
        wmix = np.concatenate([wk_w, wv_w, wq_w], axis=2)  # [128, NE, 768]
        bqd = np.ascontiguousarray(bq[qcols].reshape(NJ, P).T.astype(np.float32))
        bkd = bk[kvcols].reshape(P, 1).astype(np.float32)
        bvd = np.ascontiguousarray(
            np.broadcast_to(bv[kvcols][None, :], (P, 128))
        ).astype(np.float32)
        ball = np.concatenate([bqd, bkd, bvd], axis=1).astype(np.float32)
        ins.append(
            {
                "xT": xTc.astype(BF16),
                "wmix": np.ascontiguousarray(wmix).astype(BF16),
                "wo": _wrap_pmn(Wo[qcols, :], NJ).astype(BF16),
                "ball": ball,
            }
        )
    return ins


def gather_outputs(results, bo):
    """Sum the 4 row-parallel partials per batch, add bias."""
    y = np.zeros((B, S, E), np.float64)
    for c in range(8):
        b = c // 4
        part = results[c]["y"]  # [128, NT, E] bf16
        y[b] += part.transpose(1, 0, 2).reshape(S, E).astype(np.float64)
    return (y + bo.astype(np.float64)).astype(np.float32)


def kernel(x, Wq, bq, Wk, bk, Wv, bv, Wo, bo):
    from concourse.bass_utils import run_bass_kernel_spmd

    x = np.asarray(x, np.float32)
    nc = _get_program()
    ins = shard_inputs(
        x,
        np.asarray(Wq, np.float32),
        np.asarray(bq, np.float32),
        np.asarray(Wk, np.float32),
        np.asarray(bk, np.float32),
        np.asarray(Wv, np.float32),
        np.asarray(bv, np.float32),
        np.asarray(Wo, np.float32),
    )
    r = run_bass_kernel_spmd(nc, ins, list(range(8)), **_RUN_KWARGS)
    globals()["_last_run"] = r
    return gather_outputs(r.results, np.asarray(bo, np.float32))


# revision 46
# speedup vs baseline: 1.0016x; 1.0002x over previous
"""Grouped-Query Attention (B=2, S=2048, E=2048, 32 q heads, 8 kv heads, d=64)
on 8 Trainium2 NeuronCores.

Sharding: 8 cores = 2 batches x 4 kv-head-groups. Each core handles one batch
and 2 kv heads (= 8 q heads), computing its slice of attention plus the
row-parallel partial out-projection. The host sums the 4 partial outputs per
batch (no on-device collectives needed) and adds the output bias.

On-device pipeline per core (matmuls bf16, fp32 accumulation):
  warmup: k(0) + v(tt 0..3) + q(n=0, qi=0) et-interleaved against et-granular
  DMA (xT chunk0 on sync, fused weights on scalar) so PE is fed from ~2us ->
  scoresT = k @ qT (t on partitions) -> exp on ScalarE (scale 1/8 folded in,
  no max-subtraction: scores are O(5) for unit-variance inputs) ->
  AV^T: pav[q, d|denom] += exps_tile.T @ [v | ones]  (the ones column gives
  the softmax denominator directly, per (q, head), no cross-partition
  reduction needed) -> reciprocal + normalize (DVE) -> PE transpose back to
  [d, q] -> out-projection, software-pipelined one block behind attention.
  Last iteration: pav banks pre-zeroed by PE zeros-matmuls so the 8 AV
  chains run start-free split into an ungated (tt<12) and a gated (tt>=12)
  part around the out_proj(14) filler; per-head normalize/transpose/out-proj
  pipeline ordered to avoid tile-granular WAR serialization; final output
  pieces drain on DVE+ACT copies and sync+scalar DMA queues.
"""

import sys

sys.path.insert(0, "/opt/trn_rl_repo")

import numpy as np
import ml_dtypes

BF16 = ml_dtypes.bfloat16

P = 128
B, S, E = 2, 2048, 2048
NUM_HEADS, NUM_KV_HEADS, HEAD_DIM = 32, 8, 64
GROUP = NUM_HEADS // NUM_KV_HEADS  # 4
NE = E // P  # 16 e-tiles (contraction tiles for projections)
NT = S // P  # 16 t-tiles (key/value positions)
NJ = GROUP  # 4 q-heads per kv head
SB = 128  # query-block size
NSB = S // SB  # 16 query blocks
NCH = 4  # xT S-chunks
SCALE = 1.0 / np.sqrt(HEAD_DIM)

_compiled = None  # cached program
_RUN_KWARGS = {}  # test harness may set e.g. {"trace": True}
_last_run = None  # BassKernelResults of the most recent kernel() call
PHASE = [""]  # build-time phase label, for the analysis tooling


def build_gqa_program():
    from concourse import bacc, mybir, tile, masks

    f32 = mybir.dt.float32
    bf16 = mybir.dt.bfloat16
    Exp = mybir.ActivationFunctionType.Exp
    Copy = mybir.ActivationFunctionType.Copy

    nc = bacc.Bacc(None, target_bir_lowering=False, debug=False)
    with tile.TileContext(nc) as tc:
        with tc.tile_pool(name="dram", bufs=1, space="DRAM") as dram:
            xT = dram.tile([P, NCH, NE, 512], bf16, kind="ExternalInput", name="xT", uniquify=False)
            # wmix[:, et] = [wk 128 | wv 128 | wq j0..j3 512] for that e-tile
            wmix = dram.tile([P, NE, 768], bf16, kind="ExternalInput", name="wmix", uniquify=False)
            wo = dram.tile([P, NJ, E], bf16, kind="ExternalInput", name="wo", uniquify=False)
            # ball = [bq j0..j3 (per-partition) | bk (per-partition) | bv 128 (free-dim)]
            ball = dram.tile([P, 133], f32, kind="ExternalInput", name="ball", uniquify=False)
            y = dram.tile([P, NT, E], bf16, kind="ExternalOutput", name="y", uniquify=False)

            with (
                tc.tile_pool(name="win", bufs=1) as win,
                tc.tile_pool(name="proj", bufs=1) as proj,
                tc.tile_pool(name="attn", bufs=2) as attn,
                tc.tile_pool(name="misc", bufs=2) as misc,
                tc.tile_pool(name="ps", bufs=2, space="PSUM") as ps,
            ):
                # ---- input DMAs, 3 queues, consumption-ordered. Warmup data
                # (chunk0 + wmix) is split across all three queues; the exp
                # table load pins the scalar queue's head, so the first
                # critical pieces ride sync/gpsimd.
                xT_sb = win.tile([P, NCH, NE, 512], bf16)
                wmix_sb = win.tile([P, NE, 768], bf16)
                ball_sb = win.tile([P, 133], f32)
                wo_sb = win.tile([P, NJ, E], bf16)

                nc.sync.dma_start(out=wmix_sb[:, 0, 0:256], in_=wmix[:, 0, 0:256])
                nc.sync.dma_start(out=xT_sb[:, 0, 1:2], in_=xT[:, 0, 1:2])
                nc.sync.dma_start(out=xT_sb[:, 0, 2:4], in_=xT[:, 0, 2:4])
                nc.sync.dma_start(out=xT_sb[:, 0, 4:6], in_=xT[:, 0, 4:6])
                nc.sync.dma_start(out=xT_sb[:, 0, 6:8], in_=xT[:, 0, 6:8])
                nc.sync.dma_start(out=wmix_sb[:, 11:16], in_=wmix[:, 11:16])
                nc.sync.dma_start(out=ball_sb[:], in_=ball[:])
                nc.sync.dma_start(out=xT_sb[:, 2, 0:8], in_=xT[:, 2, 0:8])
                nc.sync.dma_start(out=xT_sb[:, 2, 8:16], in_=xT[:, 2, 8:16])

                nc.scalar.dma_start(out=wmix_sb[:, 0, 256:768], in_=wmix[:, 0, 256:768])
                nc.scalar.dma_start(out=wmix_sb[:, 1:2], in_=wmix[:, 1:2])
                nc.scalar.dma_start(out=wmix_sb[:, 2:4], in_=wmix[:, 2:4])
                nc.scalar.dma_start(out=wmix_sb[:, 7:11], in_=wmix[:, 7:11])
                nc.scalar.dma_start(out=xT_sb[:, 3, 0:8], in_=xT[:, 3, 0:8])
                nc.scalar.dma_start(out=xT_sb[:, 3, 8:16], in_=xT[:, 3, 8:16])

                nc.gpsimd.dma_start(out=xT_sb[:, 0, 0:1], in_=xT[:, 0, 0:1])
                nc.gpsimd.dma_start(out=xT_sb[:, 0, 8:12], in_=xT[:, 0, 8:12])
                nc.gpsimd.dma_start(out=wmix_sb[:, 4:7], in_=wmix[:, 4:7])
                nc.gpsimd.dma_start(out=xT_sb[:, 0, 12:16], in_=xT[:, 0, 12:16])
                # v with a ones column per kv head: [v_g | 1] -> denominator
                # comes out of the AV matmul for free.
                zrow = win.tile([1, 512], bf16)
                nc.gpsimd.memset(zrow[:], 0.0)
                v_aug = [win.tile([P, NT, 65], bf16, name=f"vaug{g}") for g in range(2)]
                for g in range(2):
                    nc.gpsimd.memset(v_aug[g][:, :, 64:65], 1.0)
                ident = win.tile([P, P], bf16)
                masks.make_identity(nc, ident[:])
                nc.gpsimd.dma_start(out=xT_sb[:, 1, 0:8], in_=xT[:, 1, 0:8])
                nc.gpsimd.dma_start(out=xT_sb[:, 1, 8:16], in_=xT[:, 1, 8:16])
                nc.gpsimd.dma_start(out=wo_sb[:], in_=wo[:])

                def wk_at(et):
                    return wmix_sb[:, et, 0:128]

                def wv_at(et):
                    return wmix_sb[:, et, 128:256]

                def wq_at(et, j):
                    return wmix_sb[:, et, 256 + j * 128:256 + (j + 1) * 128]

                bq_sb = ball_sb[:, 0:NJ]
                bk_sb = ball_sb[:, NJ:NJ + 1]
                bv_sb = ball_sb[:, NJ + 1:NJ + 1 + 128]

                # exp table warm-up; reads wmix so the ACT table load schedules
                # after the warmup-critical DMAs on the scalar queue.
                warm = misc.tile([1, 1], f32, tag="warm")
                nc.scalar.activation(out=warm[:], in_=wmix_sb[0:1, 15, 0:1], func=Exp)

                kT = proj.tile([P, S], bf16)
                qT = proj.tile([P, NJ, S], bf16)

                # ---- warmup: k(0) + v(tt 0..3) + q(n=0, qi=0) et-interleaved.
                # pvpack/pqpack hold 4 accumulation quadrants per bank; the
                # very first matmul's start=True lazily zeroes the whole bank,
                # everything else accumulates start-free.
                PHASE[0] = "warm"
                pk = ps.tile([P, 512], f32, tag="sc", name="pk")
                pvpack = ps.tile([P, 4, 128], f32, tag="sc", name="pvpack")
                pqpack = ps.tile([P, 4, 128], f32, tag="pav0", bufs=1, name="pqpack")

                def warm_q(qet):
                    for j in range(NJ):
                        nc.tensor.matmul(
                            pqpack[:, j], wq_at(qet, j), xT_sb[:, 0, qet, 0:128],
                            start=(qet == 0 and j == 0), stop=(qet == NE - 1),
                            skip_group_check=True,
                        )

                # q lags k/v by two e-tiles: its weights ride the scalar queue
                # behind the exp-table load, and the lag keeps the in-order PE
                # stream from blocking on them.
                for et in range(NE):
                    if et >= 3:
                        warm_q(et - 3)
                    nc.tensor.matmul(
                        pk[:], wk_at(et), xT_sb[:, 0, et],
                        start=(et == 0), stop=(et == NE - 1),
                    )
                    for tt in range(4):
                        nc.tensor.matmul(
                            pvpack[:, tt],
                            xT_sb[:, 0, et, tt * 128:(tt + 1) * 128],
                            wv_at(et),
                            start=(et == 0 and tt == 0), stop=(et == NE - 1),
                            skip_group_check=True,
                        )
                for qet in range(NE - 3, NE):
                    warm_q(qet)
                # pops: the first scores matmul gates on kT+qT; split the
                # bias-adds across DVE and ACT (Copy with per-partition bias)
                # so the gate chain runs on two engines in parallel.
                nc.vector.tensor_scalar_add(
                    out=kT[:, 0:512], in0=pk[:], scalar1=bk_sb
                )
                for j in (0, 1):
                    nc.vector.tensor_scalar_add(
                        out=qT[:, j, 0:128], in0=pqpack[:, j],
                        scalar1=bq_sb[:, j:j + 1],
                    )
                Ident = mybir.ActivationFunctionType.Identity
                for j in (2, 3):
                    nc.scalar.activation(
                        out=qT[:, j, 0:128], in_=pqpack[:, j], func=Ident,
                        bias=bq_sb[:, j:j + 1],
                    )
                for tt in range(4):
                    for g in range(2):
                        nc.vector.tensor_add(
                            out=v_aug[g][:, tt, 0:64],
                            in0=pvpack[:, tt, g * 64:(g + 1) * 64],
                            in1=bv_sb[:, g * 64:(g + 1) * 64],
                        )

                # ---- work units ----
                def k_proj(n):
                    pkl = ps.tile([P, 512], f32, tag="sc", name="pk")
                    for et in range(NE):
                        nc.tensor.matmul(
                            pkl[:], wk_at(et), xT_sb[:, n, et],
                            start=(et == 0), stop=(et == NE - 1),
                        )
                    nc.vector.tensor_scalar_add(
                        out=kT[:, n * 512:(n + 1) * 512], in0=pkl[:],
                        scalar1=bk_sb,
                    )

                def q_proj_quanta(n, j):
                    """q-proj unit as 4 self-contained quanta (128 cols each)."""

                    def quarter(qi):
                        def run():
                            pq = ps.tile([P, 128], f32, tag="trpy", bufs=2, name="pq")
                            cl = qi * 128
                            for et in range(NE):
                                nc.tensor.matmul(
                                    pq[:], wq_at(et, j),
                                    xT_sb[:, n, et, cl:cl + 128],
                                    start=(et == 0), stop=(et == NE - 1),
                                )
                            nc.vector.tensor_scalar_add(
                                out=qT[:, j, n * 512 + cl:n * 512 + cl + 128],
                                in0=pq[:], scalar1=bq_sb[:, j:j + 1],
                            )
                        return run

                    return [quarter(qi) for qi in range(4)]

                def v_proj(tt):
                    pv = ps.tile([P, 128], f32, tag="trpy", bufs=2, name="pv")
                    c, sl = tt // 4, (tt % 4) * 128
                    for et in range(NE):
                        nc.tensor.matmul(
                            pv[:], xT_sb[:, c, et, sl:sl + 128], wv_at(et),
                            start=(et == 0), stop=(et == NE - 1),
                        )
                    for g in range(2):
                        nc.vector.tensor_add(
                            out=v_aug[g][:, tt, 0:64],
                            in0=pv[:, g * 64:(g + 1) * 64],
                            in1=bv_sb[:, g * 64:(g + 1) * 64],
                        )

                def alloc_exps():
                    return [
                        attn.tile([P, NT, NJ, SB], bf16, tag=f"exp{g}", name=f"exp{g}")
                        for g in range(2)
                    ]

                def scores_half(sb, exps, half, popper=None):
                    """One t-half of scores + exp for query block sb."""
                    ssl = slice(sb * SB, (sb + 1) * SB)
                    for grp in range(2):
                        poff = grp * 64
                        sc = ps.tile([P, 2, NJ, SB], f32, tag="sc", name="sc")
                        for q in range(2):
                            tt = 2 * half + q
                            nc.tensor.matmul(
                                sc[:, q],
                                kT[poff:poff + 64, tt * 128:(tt + 1) * 128],
                                qT[poff:poff + 64, :, ssl],
                                start=True, stop=True,
                            )
                        nc.scalar.activation(
                            out=exps[grp][:, 2 * half:2 * half + 2],
                            in_=sc[:], func=Exp, scale=float(SCALE),
                        )
                        if popper is not None:
                            popper()

                def av_chunk(pav, exps, grp, j):
                    """One (grp, head) accumulation group: full t contraction.
                    Groups must run one-at-a-time per PSUM bank."""
                    for tt in range(NT):
                        nc.tensor.matmul(
                            pav[grp][:, j],
                            exps[grp][:, tt, j],
                            v_aug[grp][:, tt],
                            start=(tt == 0), stop=(tt == NT - 1),
                            skip_group_check=True,
                        )

                def normalize_muls(pav):
                    """DVE-only: 1/denom + scale; frees the pav PSUM tiles."""
                    recs = []
                    for grp in range(2):
                        rec = misc.tile([P, NJ], f32, tag=f"rec{grp}", bufs=2, name="rec")
                        nc.vector.reciprocal(out=rec[:], in_=pav[grp][:, :, 64:65])
                        recs.append(rec)
                    aosbs = []
                    for j in range(NJ):
                        aosb = misc.tile([P, 2, 64], bf16, tag="aosb", bufs=8, name="aosb")
                        for grp in range(2):
                            nc.vector.tensor_scalar_mul(
                                out=aosb[:, grp], in0=pav[grp][:, j, 0:64],
                                scalar1=recs[grp][:, j:j + 1],
                            )
                        aosbs.append(aosb)
                    return aosbs

                def transpose_quantum(aosbs, aoTt):
                    for j in range(NJ):
                        tr = ps.tile([P, SB], bf16, tag="trpy", bufs=2, name="tr")
                        nc.tensor.transpose(tr[:], aosbs[j][:], ident[:])
                        nc.vector.tensor_copy(out=aoTt[:, j], in_=tr[:])

                def out_proj_unit(sb, aoTt, n):
                    py = ps.tile([P, 512], f32, tag="trpy", bufs=2, name="py")
                    for j in range(NJ):
                        nc.tensor.matmul(
                            py[:], aoTt[:, j],
                            wo_sb[:, j, n * 512:(n + 1) * 512],
                            start=(j == 0), stop=(j == NJ - 1),
                        )
                    ysb = misc.tile([P, 512], bf16, tag="ysb", bufs=3, name="ysb")
                    nc.vector.tensor_copy(out=ysb[:], in_=py[:])
                    eng = nc.sync if n % 2 == 0 else nc.gpsimd
                    eng.dma_start(out=y[:, sb, n * 512:(n + 1) * 512], in_=ysb[:])

                # ---- rolling work queue ----
                workq = []
                state = {"iter": -1, "h": 0}

                def eligible(ent):
                    if ent["hold_iter"] is not None and state["iter"] < ent["hold_iter"]:
                        return False
                    if ent["av_iter"] is not None:
                        if state["iter"] <= ent["av_iter"] and state["h"] < 1:
                            return False
                        return not any(e["before_av"] for e in workq)
                    return True

                def pop_quanta(budget, pred=None):
                    spent = 0
                    while workq and spent < budget:
                        idx = next(
                            (k for k, e in enumerate(workq)
                             if eligible(e) and (pred is None or pred(e))),
                            None,
                        )
                        if idx is None:
                            break
                        e = workq.pop(idx)
                        e["fn"]()
                        spent += e["cost"]

                def flush(pred):
                    for e in [e for e in workq if pred(e)]:
                        workq.remove(e)
                        e["fn"]()

                def push(cost, fn, av_iter=None, deadline=None, before_av=False,
                         hold_iter=None):
                    workq.append(
                        dict(cost=cost, fn=fn, av_iter=av_iter, deadline=deadline,
                             before_av=before_av, hold_iter=hold_iter)
                    )

                def q_block(b):
                    """quanta producing qT columns for scores block b."""
                    n, qi = b // 4, b % 4
                    return [q_proj_quanta(n, j)[qi] for j in range(NJ)]

                # ---- prologue: warmup covered k(0)/q(block 0)/v(0..3); the
                # remaining q(n=0) quanta and v units are fillers. Bridge the
                # DVE bias-adds with a few pops, then scores(0) with kT(1..3)
                # just-in-time. ----
                for b in range(1, 4):
                    for fn in q_block(b):
                        push(860, fn, deadline=b)
                for tt in range(4, NT):
                    push(880, (lambda tt=tt: v_proj(tt)), before_av=True)
                exps_cur = alloc_exps()
                # bridge the warmup bias-add chain with v units only (their
                # v_aug writes don't gate the first scores matmuls; q quanta
                # would add qT writes the sc matmuls wait on)
                pop_quanta(2600, pred=lambda e: e["before_av"])
                for h in range(8):
                    if h in (2, 4, 6):
                        k_proj(h // 2)
                    scores_half(0, exps_cur, h,
                                (lambda: pop_quanta(250)) if h < 2 else None)

                # ---- software-pipelined main loop ----
                ao_prev = None
                q_pushed = set()
                for i in range(NSB):
                    PHASE[0] = f"it{i:02d}"
                    state["iter"], state["h"] = i, 0
                    has_next = i + 1 < NSB
                    exps_next = alloc_exps() if has_next else None
                    ecur = exps_cur
                    if has_next:
                        pav = [
                            ps.tile([P, NJ, 65], f32, tag=f"pav{g}", bufs=1, name=f"pav{g}")
                            for g in range(2)
                        ]
                        for g in range(2):
                            for j in range(NJ):
                                push(
                                    450,
                                    (lambda g=g, j=j, pav=pav, e=ecur: av_chunk(pav, e, g, j)),
                                    av_iter=i,
                                )
                    if ao_prev is not None:
                        ao = ao_prev
                        last_op = None
                        for n in range(4):
                            fn = (lambda n=n, ao=ao, s=i - 1: out_proj_unit(s, ao, n))
                            if i == NSB - 1 and n == 3:
                                last_op = fn  # emitted between av2s and the nm chain
                            else:
                                push(860, fn, hold_iter=None)
                    b = i + 2
                    if 4 <= b <= NSB - 1:
                        for fn in q_block(b):
                            push(860, fn, deadline=b,
                                 hold_iter=(14 if b == 15 else None))

                    if has_next:
                        flush(lambda e: e["deadline"] is not None and e["deadline"] <= i + 1)
                        for h in range(8):
                            state["h"] = h
                            scores_half(i + 1, exps_next, h, lambda: pop_quanta(700))
                        state["h"] = 8
                        flush(lambda e: e["before_av"])
                        flush(lambda e: e["av_iter"] == i)
                        aosbs = normalize_muls(pav)
                        aoTt = attn.tile([P, NJ, SB], bf16, tag="aoT", bufs=2, name="aoTt")
                        push(300, (lambda a=aosbs, t=aoTt: transpose_quantum(a, t)))
                        ao_prev = aoTt
                        exps_cur = exps_next
                        continue

                    # ---- last iteration (i == 15) ----
                    # pav banks pre-zeroed by PE zeros-matmuls -> the 8 AV
                    # chains accumulate start-free, split into an ungated part
                    # (tt<12) after the out_proj(14) filler and a gated tail
                    # (tt>=12, waits on ACT's trailing exps).
                    PHASE[0] = "ep"
                    state["h"] = 8
                    # pav split into lo (j0,j1) / hi (j2,j3) tiles, each holding
                    # both groups in one bank: the nm chain for j0/j1 then only
                    # waits the lo AV writes, starting the transpose/out-proj
                    # pipeline two AV chains earlier.
                    pav_lo = ps.tile([P, 2, 2, 65], f32, tag="pav0", bufs=1, name="pavlo")
                    pav_hi = ps.tile([P, 2, 2, 65], f32, tag="pav1", bufs=1, name="pavhi")
                    # pre-zero on DVE (idle here) instead of PE zeros-matmuls:
                    # keeps the 216ns off the PE critical path. DVE may write
                    # PSUM (unlike GPSIMD).
                    for t_ in (pav_lo, pav_hi):
                        nc.vector.memset(t_[:], 0.0)

                    def pav_at(grp, j):
                        t_ = pav_lo if j < 2 else pav_hi
                        return t_[:, grp, j % 2]
                    # ungated filler: tr(14) + out_proj(14)
                    flush(lambda e: e["av_iter"] is None)

                    def av_part(grp, j, t0, t1):
                        for tt in range(t0, t1):
                            nc.tensor.matmul(
                                pav_at(grp, j),
                                ecur[grp][:, tt, j],
                                v_aug[grp][:, tt],
                                start=False, stop=(tt == NT - 1),
                                skip_group_check=True,
                            )

                    for j in range(NJ):
                        for g in range(2):
                            av_part(g, j, 0, 12)
                    for j in range(NJ):
                        for g in range(2):
                            av_part(g, j, 12, NT)
                    if last_op is not None:
                        # out_proj(14, n3): fills PE while DVE runs the nm
                        # chain; its drain rides ACT+Pool to keep DVE clear.
                        py14 = ps.tile([P, 512], f32, tag="sc", name="py14")
                        for j in range(NJ):
                            nc.tensor.matmul(
                                py14[:], ao_prev[:, j],
                                wo_sb[:, j, 3 * 512:4 * 512],
                                start=(j == 0), stop=(j == NJ - 1),
                            )
                        ysb14 = misc.tile([P, 512], bf16, tag="ysb", bufs=3, name="ysb")
                        nc.scalar.activation(out=ysb14[:], in_=py14[:], func=Copy)
                        nc.gpsimd.dma_start(out=y[:, NSB - 2, 3 * 512:4 * 512], in_=ysb14[:])
                    # per-head normalize (DVE reads pav after all AV writes ->
                    # no tile-WAR backpressure on PE)
                    aosbs = []
                    for j in range(NJ):
                        aosb = misc.tile([P, 2, 64], bf16, tag="aosb", bufs=8, name="aosbl")
                        for g in range(2):
                            rec = misc.tile([P, 1], f32, tag="recl", bufs=4, name="recl")
                            nc.vector.reciprocal(out=rec[:], in_=pav_at(g, j)[:, 64:65])
                            nc.vector.tensor_scalar_mul(
                                out=aosb[:, g], in0=pav_at(g, j)[:, 0:64],
                                scalar1=rec[:, 0:1],
                            )
                        aosbs.append(aosb)
                    # transpose + out-proj pieces, interleaved so PE never sits
                    # behind a DVE copy in program order.
                    # four independent piece tiles (two sc slots + the pav
                    # slots, which free right as the pieces start) -> four
                    # parallel one-copy drain chains.
                    pyA = ps.tile([P, 512], f32, tag="sc", name="pyA")
                    pyB = ps.tile([P, 512], f32, tag="sc", name="pyB")
                    pyC = ps.tile([P, 512], f32, tag="pav0", bufs=1, name="pyC")
                    pyD = ps.tile([P, 512], f32, tag="pav1", bufs=1, name="pyD")
                    aoT15 = attn.tile([P, NJ, SB], bf16, tag="aoT", bufs=2, name="aoT15")
                    # B last: its PSUM slot frees only after out_proj(14,n3)
                    # drains, so its j0 matmul must trail the others.
                    pieces = [
                        ("A", lambda: pyA[:], 0, 512, "j0"),
                        ("C", lambda: pyC[:], 1024, 512, "j0"),
                        ("D", lambda: pyD[:], 1536, 512, "j0"),
                        ("B", lambda: pyB[:], 512, 512, "j0"),
                    ]
                    trs = []

                    def tr_j(j):
                        tr = ps.tile([P, SB], bf16, tag="trpy", bufs=2, name="tr")
                        nc.tensor.transpose(tr[:], aosbs[j][:], ident[:])
                        trs.append(tr)

                    def copy_j(j):
                        nc.vector.tensor_copy(out=aoT15[:, j], in_=trs[j][:])

                    def pieces_j(j):
                        for _, ap, col, w, st in pieces:
                            nc.tensor.matmul(
                                ap(), aoT15[:, j],
                                wo_sb[:, j, col:col + w],
                                start=(st == "j0" and j == 0), stop=(j == NJ - 1),
                                skip_group_check=True,
                            )

                    tr_j(0)
                    tr_j(1)
                    copy_j(0)
                    pieces_j(0)
                    tr_j(2)
                    copy_j(1)
                    pieces_j(1)
                    tr_j(3)
                    copy_j(2)
                    pieces_j(2)
                    copy_j(3)
                    pieces_j(3)
                    # drain: copies spread over DVE/ACT/Pool, DMAs over
                    # sync/scalar/gpsimd; all piece matmuls precede all copies,
                    # so no WAR dependency can stall the PE.
                    # one copy per tile -> four parallel chains on
                    # DVE/Pool/ACT/Pool with DMAs spread over all queues.
                    # GPSIMD cannot read PSUM on real HW: copies go on
                    # DVE/ACT only, alternating so each engine's chain is short.
                    cp_eng = [
                        nc.vector.tensor_copy,        # A
                        None,                         # C -> ACT Copy
                        nc.vector.tensor_copy,        # D
                        None,                         # B -> ACT Copy
                    ]
                    dma_eng = [nc.sync, nc.gpsimd, nc.sync, nc.scalar]
                    for pi, (_, ap, col, w, _st) in enumerate(pieces):
                        ysb = misc.tile([P, w], bf16, tag="ydr", bufs=5, name="ydr")
                        if cp_eng[pi] is None:
                            nc.scalar.activation(out=ysb[:], in_=ap(), func=Copy)
                        else:
                            cp_eng[pi](out=ysb[:], in_=ap())
                        dma_eng[pi].dma_start(
                            out=y[:, NSB - 1, col:col + w], in_=ysb[:]
                        )
                flush(lambda e: True)
    nc.compile()
    return nc


def _get_program():
    global _compiled
    if _compiled is None:
        _compiled = build_gqa_program()
    return _compiled


def _wrap_pmn(a2d, ntile):
    """[R, C] -> [128, R/128, C] with row r at (r % 128, r // 128)."""
    r, c = a2d.shape
    return np.ascontiguousarray(a2d.reshape(ntile, P, c).transpose(1, 0, 2))


def shard_inputs(x, Wq, bq, Wk, bk, Wv, bv, Wo):
    """Build the 8 per-core input maps (host-side shard + transpose + cast)."""
    ins = []
    for c in range(8):
        b, g = c // 4, c % 4
        # q-head columns for this core, ordered (j, pair, d):
        # global q-col = (2g + pair) * 256 + j * 64 + d
        j_idx, pair_idx, d_idx = np.meshgrid(
            np.arange(NJ), np.arange(2), np.arange(64), indexing="ij"
        )
        qcols = ((2 * g + pair_idx) * (GROUP * 64) + j_idx * 64 + d_idx).reshape(-1)
        kvcols = np.arange(g * 128, (g + 1) * 128)  # kv heads 2g, 2g+1

        xT = np.ascontiguousarray(x[b].T)  # [E, S] f32
        xTw = _wrap_pmn(xT, NE)  # [128, NE, S]
        xTc = np.ascontiguousarray(
            xTw.reshape(P, NE, NCH, 512).transpose(0, 2, 1, 3)
        )  # [128, NCH, NE, 512]
        wk_w = _wrap_pmn(Wk[:, kvcols], NE)  # [128, NE, 128]
        wv_w = _wrap_pmn(Wv[:, kvcols], NE)  # [128, NE, 128]
        wq_w = _wrap_pmn(Wq[:, qcols], NE)  # [128, NE, 512]
        wmix = np.concatenate([wk_w, wv_w, wq_w], axis=2)  # [128, NE, 768]
        bqd = np.ascontiguousarray(bq[qcols].reshape(NJ, P).T.astype(np.float32))
        bkd = bk[kvcols].reshape(P, 1).astype(np.float32)
        bvd = np.ascontiguousarray(
            np.broadcast_to(bv[kvcols][None, :], (P, 128))
        ).astype(np.float32)
        ball = np.concatenate([bqd, bkd, bvd], axis=1).astype(np.float32)
        ins.append(
            {
                "xT": xTc.astype(BF16),
                "wmix": np.ascontiguousarray(wmix).astype(BF16),
                "wo": _wrap_pmn(Wo[qcols, :], NJ).astype(BF16),
                "ball": ball,
            }
        )
    return ins


def gather_outputs(results, bo):
    """Sum the 4 row-parallel partials per batch, add bias."""
    y = np.zeros((B, S, E), np.float64)
    for c in range(8):
        b = c // 4
        part = results[c]["y"]  # [128, NT, E] bf16
        y[b] += part.transpose(1, 0, 2).reshape(S, E).astype(np.float64)
    return (y + bo.astype(np.float64)).astype(np.float32)


def kernel(x, Wq, bq, Wk, bk, Wv, bv, Wo, bo):
    from concourse.bass_utils import run_bass_kernel_spmd

    x = np.asarray(x, np.float32)
    nc = _get_program()
    ins = shard_inputs(
        x,
        np.asarray(Wq, np.float32),
        np.asarray(bq, np.float32),
        np.asarray(Wk, np.float32),
        np.asarray(bk, np.float32),
        np.asarray(Wv, np.float32),
        np.asarray(bv, np.float32),
        np.asarray(Wo, np.float32),
    )
    r = run_bass_kernel_spmd(nc, ins, list(range(8)), **_RUN_KWARGS)
    globals()["_last_run"] = r
    return gather_outputs(r.results, np.asarray(bo, np.float32))
